# revision 3
# baseline (speedup 1.0000x reference)
"""AttentionHead kernel for Trainium2, 8 NeuronCores.

Sharding: core c -> (batch b = c//2, query-half h = c%2).
Each core computes K/V projections over the full 4096-token sequence of its
batch and Q projections + attention for its 2048-token query half.

Host-side prep: hidden_state[b] is transposed to xT = [EMBED, SEQ] so the
contraction dim (EMBED) lands on SBUF partitions with no on-chip transposes.

On-chip layout (per core):
  kvT  [128, 4096] sbuf fp16: rows 0:64 = K^T (D x Tk), rows 64:128 = V^T
  qT   [64, 2048]  sbuf fp16: Q^T for own query half
  vaug [128, 32*65] sbuf fp16: 32 chunks of V [128 tokens, 64] + ones column
  scores computed transposed: S^T[tk, tq] = (K^T chunk).T @ Q^T  (psum f32)
  exp on ScalarE with fused 1/sqrt(64) scale (no max subtraction: |s|<~3)
  AV: out^T_aug[65, tq] += vaug[chunk].T @ expS^T chunk
      row 64 accumulates the softmax denominator.
Output per core: [65, 2048] f32; host divides rows 0:64 by row 64, transposes.

All matmul operands are fp16 (cast in-flight by SWDGE DMA / on PSUM->SBUF
copies); accumulation is fp32 in PSUM.
"""

import os
import numpy as np

EMBED = 1024
SEQ = 4096
TQ = 2048  # query tokens per core
D = 64
NT = 512  # token chunk (free dim) for projections
P = 128
N_CORES = 8

_CACHE = {}
LAST_RESULTS = None


def _build_bass():
    import concourse.bass as bass
    import concourse.mybir as mybir
    import concourse.tile as tile
    from concourse import bacc

    f32 = mybir.dt.float32
    f16 = mybir.dt.float16
    EXP = mybir.ActivationFunctionType.Exp

    nc = bacc.Bacc("TRN2", target_bir_lowering=False, debug=False)

    xT = nc.dram_tensor("xT", [EMBED, SEQ], f32, kind="ExternalInput").ap()
    xTq = nc.dram_tensor("xTq", [EMBED, TQ], f32, kind="ExternalInput").ap()
    wq = nc.dram_tensor("wq", [EMBED, D], f32, kind="ExternalInput").ap()
    wk = nc.dram_tensor("wk", [EMBED, D], f32, kind="ExternalInput").ap()
    wv = nc.dram_tensor("wv", [EMBED, D], f32, kind="ExternalInput").ap()
    bq = nc.dram_tensor("bq", [1, D], f32, kind="ExternalInput").ap()
    bk = nc.dram_tensor("bk", [1, D], f32, kind="ExternalInput").ap()
    bv = nc.dram_tensor("bv", [1, D], f32, kind="ExternalInput").ap()
    ident = nc.dram_tensor("ident", [D, D], f32, kind="ExternalInput").ap()
    out = nc.dram_tensor("out", [D + 1, TQ], f32, kind="ExternalOutput").ap()

    NE = EMBED // P  # 8 embed chunks
    NTG = SEQ // NT  # 8 token groups
    NKC = SEQ // P  # 32 key chunks
    NQC = TQ // NT  # 4 query chunks

    with tile.TileContext(nc) as tc:
        with tc.tile_pool(name="const", bufs=1) as const:
            wq_sb = const.tile([P, NE, D], f16, tag="wq")
            wk_sb = const.tile([P, NE, D], f16, tag="wk")
            wv_sb = const.tile([P, NE, D], f16, tag="wv")
            nc.gpsimd.dma_start(wq_sb[:], wq.rearrange("(c p) d -> p c d", p=P))
            nc.gpsimd.dma_start(wk_sb[:], wk.rearrange("(c p) d -> p c d", p=P))
            nc.gpsimd.dma_start(wv_sb[:], wv.rearrange("(c p) d -> p c d", p=P))
            bq_sb = const.tile([1, D], f16, tag="bq")
            bk_sb = const.tile([1, D], f16, tag="bk")
            bv_sb = const.tile([1, D], f16, tag="bv")
            nc.gpsimd.dma_start(bq_sb[:], bq[:])
            nc.gpsimd.dma_start(bk_sb[:], bk[:])
            nc.gpsimd.dma_start(bv_sb[:], bv[:])
            id_sb = const.tile([D, D], f16, tag="ident")
            nc.gpsimd.dma_start(id_sb[:], ident[:])
            ones_sb = const.tile([1, NT], f16, tag="ones")
            nc.gpsimd.memset(ones_sb[:], 1.0)

            kT = const.tile([D, SEQ], f16, tag="kT")
            vT = const.tile([D, SEQ], f16, tag="vT")
            qT = const.tile([D, TQ], f16, tag="qT")
            vaug = const.tile([P, NKC * (D + 1)], f16, tag="vaug")
            # ones column of each vaug chunk
            nc.gpsimd.memset(
                vaug[:].rearrange("p (c w) -> p c w", w=D + 1)[:, :, D : D + 1], 1.0
            )

            # ---------------- projection phase ----------------
            with (
                tc.tile_pool(name="xg", bufs=2) as xgp,
                tc.tile_pool(name="ppsum", bufs=2, space="PSUM") as pps,
            ):
                for tg in range(NTG):
                    xg = xgp.tile([P, NE, NT], f16, tag="xg")
                    nc.gpsimd.dma_start(
                        xg[:],
                        xT[:, tg * NT : (tg + 1) * NT].rearrange(
                            "(c p) t -> p c t", p=P
                        ),
                    )
                    pk = pps.tile([D, NT], f32, tag="pk")
                    pv = pps.tile([D, NT], f32, tag="pv")
                    for e in range(NE):
                        nc.tensor.matmul(
                            pk[:, :],
                            wk_sb[:, e, :],
                            xg[:, e, :],
                            start=(e == 0),
                            stop=False,
                        )
                    nc.tensor.matmul(
                        pk[:, :], bk_sb[:], ones_sb[:], start=False, stop=True
                    )
                    for e in range(NE):
                        nc.tensor.matmul(
                            pv[:, :],
                            wv_sb[:, e, :],
                            xg[:, e, :],
                            start=(e == 0),
                            stop=False,
                        )
                    nc.tensor.matmul(
                        pv[:, :], bv_sb[:], ones_sb[:], start=False, stop=True
                    )
                    nc.scalar.copy(out=kT[:, tg * NT : (tg + 1) * NT], in_=pk[:, :])
                    nc.vector.tensor_copy(
                        out=vT[:, tg * NT : (tg + 1) * NT], in_=pv[:, :]
                    )

                # Q projection over own half (xTq), 4 groups of 512
                for tg in range(NQC):
                    xq = xgp.tile([P, NE, NT], f16, tag="xg")
                    nc.gpsimd.dma_start(
                        xq[:],
                        xTq[:, tg * NT : (tg + 1) * NT].rearrange(
                            "(c p) t -> p c t", p=P
                        ),
                    )
                    pq = pps.tile([D, NT], f32, tag="pq")
                    for e in range(NE):
                        nc.tensor.matmul(
                            pq[:, :],
                            wq_sb[:, e, :],
                            xq[:, e, :],
                            start=(e == 0),
                            stop=False,
                        )
                    nc.tensor.matmul(
                        pq[:, :], bq_sb[:], ones_sb[:], start=False, stop=True
                    )
                    nc.vector.tensor_copy(
                        out=qT[:, tg * NT : (tg + 1) * NT], in_=pq[:, :]
                    )

                # V transposes: vT -> vaug chunks
                for i in range(NKC):
                    pvt = pps.tile([P, D], f16, tag="pvt")
                    nc.tensor.transpose(
                        out=pvt[:, :],
                        in_=vT[:, i * P : (i + 1) * P],
                        identity=id_sb[:, :],
                    )
                    w0 = i * (D + 1)
                    if i % 2 == 0:
                        nc.vector.tensor_copy(out=vaug[:, w0 : w0 + D], in_=pvt[:, :])
                    else:
                        nc.scalar.copy(out=vaug[:, w0 : w0 + D], in_=pvt[:, :])

            # ---------------- attention phase ----------------
            with (
                tc.tile_pool(name="apsum", bufs=1, space="PSUM") as aps,
                tc.tile_pool(name="expp", bufs=3) as expp,
                tc.tile_pool(name="outp", bufs=2) as outp,
            ):
                for qc in range(NQC):
                    pav = aps.tile([D + 1, NT], f32, tag="av", bufs=2)
                    for g in range(NKC // 2):
                        psc = aps.tile([P, 2 * NT], f32, tag="sc", bufs=3)
                        for j in range(2):
                            i = 2 * g + j
                            nc.tensor.matmul(
                                psc[:, j * NT : (j + 1) * NT],
                                kT[:, i * P : (i + 1) * P],
                                qT[:, qc * NT : (qc + 1) * NT],
                                start=True,
                                stop=True,
                            )
                        ex = expp.tile([P, 2 * NT], f16, tag="ex")
                        nc.scalar.activation(ex[:], psc[:], EXP, scale=0.125)
                        for j in range(2):
                            i = 2 * g + j
                            w0 = i * (D + 1)
                            nc.tensor.matmul(
                                pav[:, :],
                                vaug[:, w0 : w0 + D + 1],
                                ex[:, j * NT : (j + 1) * NT],
                                start=(i == 0),
                                stop=(i == NKC - 1),
                                skip_group_check=True,
                            )
                    osb = outp.tile([D + 1, NT], f32, tag="osb")
                    nc.vector.tensor_copy(out=osb[:], in_=pav[:])
                    nc.sync.dma_start(out[:, qc * NT : (qc + 1) * NT], osb[:])

    nc.compile()
    return nc


def kernel(hidden_state, q_w, q_b, k_w, k_b, v_w, v_b):
    global LAST_RESULTS
    from concourse.bass_utils import run_bass_kernel_spmd

    hidden_state = np.asarray(hidden_state, dtype=np.float32)
    q_w = np.asarray(q_w, dtype=np.float32)
    q_b = np.asarray(q_b, dtype=np.float32)
    k_w = np.asarray(k_w, dtype=np.float32)
    k_b = np.asarray(k_b, dtype=np.float32)
    v_w = np.asarray(v_w, dtype=np.float32)
    v_b = np.asarray(v_b, dtype=np.float32)

    B, S, E = hidden_state.shape
    assert (B, S, E) == (4, SEQ, EMBED)

    if "nc" not in _CACHE:
        _CACHE["nc"] = _build_bass()
    nc = _CACHE["nc"]

    ident = np.eye(D, dtype=np.float32)
    shared = {
        "wq": q_w,
        "wk": k_w,
        "wv": v_w,
        "bq": q_b.reshape(1, D).astype(np.float32),
        "bk": k_b.reshape(1, D).astype(np.float32),
        "bv": v_b.reshape(1, D).astype(np.float32),
        "ident": ident,
    }
    xTs = [np.ascontiguousarray(hidden_state[b].T) for b in range(B)]
    in_maps = []
    for c in range(N_CORES):
        b, h = c // 2, c % 2
        m = dict(shared)
        m["xT"] = xTs[b]
        m["xTq"] = np.ascontiguousarray(xTs[b][:, h * TQ : (h + 1) * TQ])
        in_maps.append(m)

    trace = bool(int(os.environ.get("KERNEL_TRACE", "0")))
    res = run_bass_kernel_spmd(nc, in_maps, list(range(N_CORES)), trace=trace)
    LAST_RESULTS = res

    outp = np.empty((B, S, D), dtype=np.float32)
    for c in range(N_CORES):
        b, h = c // 2, c % 2
        r = res.results[c]["out"]  # [65, 2048]
        outp[b, h * TQ : (h + 1) * TQ, :] = (r[:D] / r[D : D + 1]).T
    return outp


# revision 6
# speedup vs baseline: 27.0146x; 27.0146x over previous
"""AttentionHead kernel for Trainium2, 8 NeuronCores.

Sharding: core c -> (batch b = c//2, query-half h = c%2).
Each core computes K/V projections over the full 4096-token sequence of its
batch and Q projections + attention for its 2048-token query half.

Host-side prep: hidden_state[b] is transposed to xT = [EMBED, SEQ] so the
contraction dim (EMBED) lands on SBUF partitions with no on-chip transposes.

On-chip layout (per core):
  kvT  [128, 4096] sbuf fp16: rows 0:64 = K^T (D x Tk), rows 64:128 = V^T
  qT   [64, 2048]  sbuf fp16: Q^T for own query half
  vaug [128, 32*65] sbuf fp16: 32 chunks of V [128 tokens, 64] + ones column
  scores computed transposed: S^T[tk, tq] = (K^T chunk).T @ Q^T  (psum f32)
  exp on ScalarE with fused 1/sqrt(64) scale (no max subtraction: |s|<~3)
  AV: out^T_aug[65, tq] += vaug[chunk].T @ expS^T chunk
      row 64 accumulates the softmax denominator.
Output per core: [65, 2048] f32; host divides rows 0:64 by row 64, transposes.

All matmul operands are fp16 (cast in-flight by SWDGE DMA / on PSUM->SBUF
copies); accumulation is fp32 in PSUM.
"""

import os
import numpy as np

EMBED = 1024
SEQ = 4096
TQ = 2048  # query tokens per core
D = 64
NT = 512  # token chunk (free dim) for projections
P = 128
N_CORES = 8

_CACHE = {}
LAST_RESULTS = None


def _build_bass(repeats=1):
    import concourse.bass as bass
    import concourse.mybir as mybir
    import concourse.tile as tile
    from concourse import bacc

    f32 = mybir.dt.float32
    f16 = mybir.dt.float16
    EXP = mybir.ActivationFunctionType.Exp

    nc = bacc.Bacc("TRN2", target_bir_lowering=False, debug=False)

    xT = nc.dram_tensor("xT", [EMBED, SEQ], f32, kind="ExternalInput").ap()
    xTq = nc.dram_tensor("xTq", [EMBED, TQ], f32, kind="ExternalInput").ap()
    wq = nc.dram_tensor("wq", [EMBED, D], f32, kind="ExternalInput").ap()
    wk = nc.dram_tensor("wk", [EMBED, D], f32, kind="ExternalInput").ap()
    wv = nc.dram_tensor("wv", [EMBED, D], f32, kind="ExternalInput").ap()
    bq = nc.dram_tensor("bq", [1, D], f32, kind="ExternalInput").ap()
    bk = nc.dram_tensor("bk", [1, D], f32, kind="ExternalInput").ap()
    bv = nc.dram_tensor("bv", [1, D], f32, kind="ExternalInput").ap()
    ident = nc.dram_tensor("ident", [D, D], f32, kind="ExternalInput").ap()
    out = nc.dram_tensor("out", [D + 1, TQ], f32, kind="ExternalOutput").ap()

    NE = EMBED // P  # 8 embed chunks
    NTG = SEQ // NT  # 8 token groups
    NKC = SEQ // P  # 32 key chunks
    NQC = TQ // NT  # 4 query chunks

    with tile.TileContext(nc) as tc:
        with tc.tile_pool(name="const", bufs=1) as const:
            wq_sb = const.tile([P, NE, D], f16, tag="wq")
            wk_sb = const.tile([P, NE, D], f16, tag="wk")
            wv_sb = const.tile([P, NE, D], f16, tag="wv")
            nc.gpsimd.dma_start(wq_sb[:], wq.rearrange("(c p) d -> p c d", p=P))
            nc.gpsimd.dma_start(wk_sb[:], wk.rearrange("(c p) d -> p c d", p=P))
            nc.gpsimd.dma_start(wv_sb[:], wv.rearrange("(c p) d -> p c d", p=P))
            bq_sb = const.tile([1, D], f16, tag="bq")
            bk_sb = const.tile([1, D], f16, tag="bk")
            bv_sb = const.tile([1, D], f16, tag="bv")
            nc.gpsimd.dma_start(bq_sb[:], bq[:])
            nc.gpsimd.dma_start(bk_sb[:], bk[:])
            nc.gpsimd.dma_start(bv_sb[:], bv[:])
            id_sb = const.tile([D, D], f16, tag="ident")
            nc.gpsimd.dma_start(id_sb[:], ident[:])
            ones_sb = const.tile([1, NT], f16, tag="ones")
            nc.gpsimd.memset(ones_sb[:], 1.0)

            kT = const.tile([D, SEQ], f16, tag="kT")
            vT = const.tile([D, SEQ], f16, tag="vT")
            qT = const.tile([D, TQ], f16, tag="qT")
            vaug = const.tile([P, NKC * (D + 1)], f16, tag="vaug")
            # ones column of each vaug chunk
            nc.gpsimd.memset(
                vaug[:].rearrange("p (c w) -> p c w", w=D + 1)[:, :, D : D + 1], 1.0
            )

            for _rep in range(repeats):
                _kernel_body(
                    nc, tc, mybir,
                    xT, xTq, out,
                    wq_sb, wk_sb, wv_sb, bq_sb, bk_sb, bv_sb, id_sb, ones_sb,
                    kT, vT, qT, vaug,
                )

    nc.compile()
    return nc


def _kernel_body(
    nc, tc, mybir,
    xT, xTq, out,
    wq_sb, wk_sb, wv_sb, bq_sb, bk_sb, bv_sb, id_sb, ones_sb,
    kT, vT, qT, vaug,
):
    f32 = mybir.dt.float32
    f16 = mybir.dt.float16
    EXP = mybir.ActivationFunctionType.Exp
    NE = EMBED // P
    NTG = SEQ // NT
    NKC = SEQ // P
    NQC = TQ // NT
    if True:
        if True:
            # ---------------- projection phase ----------------
            with (
                tc.tile_pool(name="xg", bufs=2) as xgp,
                tc.tile_pool(name="ppsum", bufs=2, space="PSUM") as pps,
            ):
                for tg in range(NTG):
                    xg = xgp.tile([P, NE, NT], f16, tag="xg")
                    nc.gpsimd.dma_start(
                        xg[:],
                        xT[:, tg * NT : (tg + 1) * NT].rearrange(
                            "(c p) t -> p c t", p=P
                        ),
                    )
                    pk = pps.tile([D, NT], f32, tag="pk")
                    pv = pps.tile([D, NT], f32, tag="pv")
                    for e in range(NE):
                        nc.tensor.matmul(
                            pk[:, :],
                            wk_sb[:, e, :],
                            xg[:, e, :],
                            start=(e == 0),
                            stop=False,
                        )
                    nc.tensor.matmul(
                        pk[:, :], bk_sb[:], ones_sb[:], start=False, stop=True
                    )
                    for e in range(NE):
                        nc.tensor.matmul(
                            pv[:, :],
                            wv_sb[:, e, :],
                            xg[:, e, :],
                            start=(e == 0),
                            stop=False,
                        )
                    nc.tensor.matmul(
                        pv[:, :], bv_sb[:], ones_sb[:], start=False, stop=True
                    )
                    nc.scalar.copy(out=kT[:, tg * NT : (tg + 1) * NT], in_=pk[:, :])
                    nc.vector.tensor_copy(
                        out=vT[:, tg * NT : (tg + 1) * NT], in_=pv[:, :]
                    )

                # Q projection over own half (xTq), 4 groups of 512
                for tg in range(NQC):
                    xq = xgp.tile([P, NE, NT], f16, tag="xg")
                    nc.gpsimd.dma_start(
                        xq[:],
                        xTq[:, tg * NT : (tg + 1) * NT].rearrange(
                            "(c p) t -> p c t", p=P
                        ),
                    )
                    pq = pps.tile([D, NT], f32, tag="pq")
                    for e in range(NE):
                        nc.tensor.matmul(
                            pq[:, :],
                            wq_sb[:, e, :],
                            xq[:, e, :],
                            start=(e == 0),
                            stop=False,
                        )
                    nc.tensor.matmul(
                        pq[:, :], bq_sb[:], ones_sb[:], start=False, stop=True
                    )
                    nc.vector.tensor_copy(
                        out=qT[:, tg * NT : (tg + 1) * NT], in_=pq[:, :]
                    )

                # V transposes: vT -> vaug chunks
                for i in range(NKC):
                    pvt = pps.tile([P, D], f16, tag="pvt")
                    nc.tensor.transpose(
                        out=pvt[:, :],
                        in_=vT[:, i * P : (i + 1) * P],
                        identity=id_sb[:, :],
                    )
                    w0 = i * (D + 1)
                    if i % 2 == 0:
                        nc.vector.tensor_copy(out=vaug[:, w0 : w0 + D], in_=pvt[:, :])
                    else:
                        nc.scalar.copy(out=vaug[:, w0 : w0 + D], in_=pvt[:, :])

            # ---------------- attention phase ----------------
            with (
                tc.tile_pool(name="apsum", bufs=1, space="PSUM") as aps,
                tc.tile_pool(name="expp", bufs=3) as expp,
                tc.tile_pool(name="outp", bufs=2) as outp,
            ):
                for qc in range(NQC):
                    pav = aps.tile([D + 1, NT], f32, tag="av", bufs=2)
                    for g in range(NKC // 2):
                        psc = aps.tile([P, 2 * NT], f32, tag="sc", bufs=3)
                        for j in range(2):
                            i = 2 * g + j
                            nc.tensor.matmul(
                                psc[:, j * NT : (j + 1) * NT],
                                kT[:, i * P : (i + 1) * P],
                                qT[:, qc * NT : (qc + 1) * NT],
                                start=True,
                                stop=True,
                            )
                        ex = expp.tile([P, 2 * NT], f16, tag="ex")
                        nc.scalar.activation(ex[:], psc[:], EXP, scale=0.125)
                        for j in range(2):
                            i = 2 * g + j
                            w0 = i * (D + 1)
                            nc.tensor.matmul(
                                pav[:, :],
                                vaug[:, w0 : w0 + D + 1],
                                ex[:, j * NT : (j + 1) * NT],
                                start=(i == 0),
                                stop=(i == NKC - 1),
                                skip_group_check=True,
                            )
                    osb = outp.tile([D + 1, NT], f32, tag="osb")
                    nc.vector.tensor_copy(out=osb[:], in_=pav[:])
                    nc.sync.dma_start(out[:, qc * NT : (qc + 1) * NT], osb[:])


def kernel(hidden_state, q_w, q_b, k_w, k_b, v_w, v_b):
    global LAST_RESULTS
    from concourse.bass_utils import run_bass_kernel_spmd

    hidden_state = np.asarray(hidden_state, dtype=np.float32)
    q_w = np.asarray(q_w, dtype=np.float32)
    q_b = np.asarray(q_b, dtype=np.float32)
    k_w = np.asarray(k_w, dtype=np.float32)
    k_b = np.asarray(k_b, dtype=np.float32)
    v_w = np.asarray(v_w, dtype=np.float32)
    v_b = np.asarray(v_b, dtype=np.float32)

    B, S, E = hidden_state.shape
    assert (B, S, E) == (4, SEQ, EMBED)

    if "nc" not in _CACHE:
        _CACHE["nc"] = _build_bass()
    nc = _CACHE["nc"]

    ident = np.eye(D, dtype=np.float32)
    shared = {
        "wq": q_w,
        "wk": k_w,
        "wv": v_w,
        "bq": q_b.reshape(1, D).astype(np.float32),
        "bk": k_b.reshape(1, D).astype(np.float32),
        "bv": v_b.reshape(1, D).astype(np.float32),
        "ident": ident,
    }
    xTs = [np.ascontiguousarray(hidden_state[b].T) for b in range(B)]
    in_maps = []
    for c in range(N_CORES):
        b, h = c // 2, c % 2
        m = dict(shared)
        m["xT"] = xTs[b]
        m["xTq"] = np.ascontiguousarray(xTs[b][:, h * TQ : (h + 1) * TQ])
        in_maps.append(m)

    trace = bool(int(os.environ.get("KERNEL_TRACE", "0")))
    res = run_bass_kernel_spmd(nc, in_maps, list(range(N_CORES)), trace=trace)
    LAST_RESULTS = res

    outp = np.empty((B, S, D), dtype=np.float32)
    for c in range(N_CORES):
        b, h = c // 2, c % 2
        r = res.results[c]["out"]  # [65, 2048]
        outp[b, h * TQ : (h + 1) * TQ, :] = (r[:D] / r[D : D + 1]).T
    return outp


# revision 12
# speedup vs baseline: 112.4547x; 4.1627x over previous
"""AttentionHead kernel for Trainium2, 8 NeuronCores.

Sharding: core c -> (batch b = c//2, query-half h = c%2).
Each core computes K/V projections over the full 4096-token sequence of its
batch and Q projections + attention for its 2048-token query half.

Host-side prep: hidden_state[b] is transposed to xT = [EMBED, SEQ] so the
contraction dim (EMBED) lands on SBUF partitions with no on-chip transposes.

v2: PE array packing + proj/attention interleaving.
 - Projections are col-paired (tile_position (0,0)/(0,64)): one matmul pair
   computes two 64-wide output blocks concurrently on the two column halves
   of the PE array.  K^T/V^T land as [128, 2048] tiles whose rows 0:64 hold
   token chunks 0:2048 and rows 64:128 hold chunks 2048:4096.
 - Scores are row-paired (tile_position (0,0)/(64,0)): contraction is D=64,
   so two independent S^T chunk matmuls share the PE array's row halves.
 - Emission interleaves: token-pair p's K/V projection + V transposes are
   followed immediately by attention g-blocks 4p..4p+3, so attention starts
   as soon as the first 4MB of activations has landed; DMA and projections
   hide behind the ACT-bound exp stream.

scores: S^T[tk, tq] = (K^T chunk).T @ Q^T   (f32 psum, pairs -> [128, 1024])
exp on ScalarE, fused 1/sqrt(64) scale (no max subtraction: |scores/8| < ~3)
AV: out^T_aug[65, tq] += vaug[chunk].T @ expS^T chunk  (vaug has ones column,
row 64 accumulates the softmax denominator).
Output per core: [65, 2048] f32; host divides rows 0:64 by row 64, transposes.

All matmul operands fp16 (SWDGE casts f32->f16 in flight); psum f32.
"""

import os
import numpy as np

EMBED = 1024
SEQ = 4096
TQ = 2048  # query tokens per core
D = 64
NT = 512  # token chunk (free dim) for projections
P = 128
N_CORES = 8

_CACHE = {}
LAST_RESULTS = None


def _build_bass(repeats=1):
    import concourse.bass as bass
    import concourse.mybir as mybir
    import concourse.tile as tile
    from concourse import bacc

    f32 = mybir.dt.float32
    f16 = mybir.dt.float16

    nc = bacc.Bacc("TRN2", target_bir_lowering=False, debug=False)

    xT = nc.dram_tensor("xT", [EMBED, SEQ], f32, kind="ExternalInput").ap()
    xTq = nc.dram_tensor("xTq", [EMBED, TQ], f32, kind="ExternalInput").ap()
    wq = nc.dram_tensor("wq", [EMBED, D], f32, kind="ExternalInput").ap()
    wk = nc.dram_tensor("wk", [EMBED, D], f32, kind="ExternalInput").ap()
    wv = nc.dram_tensor("wv", [EMBED, D], f32, kind="ExternalInput").ap()
    bq = nc.dram_tensor("bq", [1, D], f32, kind="ExternalInput").ap()
    bk = nc.dram_tensor("bk", [1, D], f32, kind="ExternalInput").ap()
    bv = nc.dram_tensor("bv", [1, D], f32, kind="ExternalInput").ap()
    ident = nc.dram_tensor("ident", [P, P], f32, kind="ExternalInput").ap()
    out = nc.dram_tensor("out", [D + 1, TQ], f32, kind="ExternalOutput").ap()

    NE = EMBED // P  # 8 embed chunks

    with tile.TileContext(nc) as tc:
        with tc.tile_pool(name="const", bufs=1) as const:
            wq_sb = const.tile([P, NE, D], f16, tag="wq")
            wk_sb = const.tile([P, NE, D], f16, tag="wk")
            wv_sb = const.tile([P, NE, D], f16, tag="wv")
            nc.gpsimd.dma_start(wq_sb[:], wq.rearrange("(c p) d -> p c d", p=P))
            nc.gpsimd.dma_start(wk_sb[:], wk.rearrange("(c p) d -> p c d", p=P))
            nc.gpsimd.dma_start(wv_sb[:], wv.rearrange("(c p) d -> p c d", p=P))
            bq_sb = const.tile([1, D], f16, tag="bq")
            bk_sb = const.tile([1, D], f16, tag="bk")
            bv_sb = const.tile([1, D], f16, tag="bv")
            nc.gpsimd.dma_start(bq_sb[:], bq[:])
            nc.gpsimd.dma_start(bk_sb[:], bk[:])
            nc.gpsimd.dma_start(bv_sb[:], bv[:])
            id_sb = const.tile([P, P], f16, tag="ident")
            nc.gpsimd.dma_start(id_sb[:], ident[:])
            ones_sb = const.tile([1, NT], f16, tag="ones")
            nc.gpsimd.memset(ones_sb[:], 1.0)

            # rows 0:64 = token chunks [0, 2048); rows 64:128 = [2048, 4096)
            kk = const.tile([P, SEQ // 2], f16, tag="kk")
            vv = const.tile([P, SEQ // 2], f16, tag="vv")
            # qq1 rows 0:64 = q chunks [0,1024); rows 64:128 = [1024, 2048)
            # qq2 = qq1 with the halves swapped (partition-shift DMA)
            qq1 = const.tile([P, TQ // 2], f16, tag="qq1")
            qq2 = const.tile([P, TQ // 2], f16, tag="qq2")
            NKC = SEQ // P  # 32 key chunks
            vaug = const.tile([P, NKC * (D + 1)], f16, tag="vaug")
            nc.gpsimd.memset(
                vaug[:].rearrange("p (c w) -> p c w", w=D + 1)[:, :, D : D + 1], 1.0
            )

            for _rep in range(repeats):
                _kernel_body(
                    nc, tc, mybir,
                    xT, xTq, out,
                    wq_sb, wk_sb, wv_sb, bq_sb, bk_sb, bv_sb, id_sb, ones_sb,
                    kk, vv, qq1, qq2, vaug,
                )

    nc.compile()
    return nc


def _kernel_body(
    nc, tc, mybir,
    xT, xTq, out,
    wq_sb, wk_sb, wv_sb, bq_sb, bk_sb, bv_sb, id_sb, ones_sb,
    kk, vv, qq1, qq2, vaug,
):
    f32 = mybir.dt.float32
    f16 = mybir.dt.float16
    EXP = mybir.ActivationFunctionType.Exp
    NE = EMBED // P
    NKC = SEQ // P
    NQC = TQ // NT  # 4 query chunks of 512

    with (
        tc.tile_pool(name="xg", bufs=4) as xgp,
        tc.tile_pool(name="psum", bufs=2, space="PSUM") as pps,
        tc.tile_pool(name="expp", bufs=3) as expp,
        tc.tile_pool(name="outp", bufs=2) as outp,
    ):
        # ---- Q projections first (gate all attention) ----
        for qp in range(2):  # pair q-group qp with qp+2
            xa = xgp.tile([P, NE, NT], f16, tag="xg")
            xb = xgp.tile([P, NE, NT], f16, tag="xg")
            nc.gpsimd.dma_start(
                xa[:],
                xTq[:, qp * NT : (qp + 1) * NT].rearrange("(c p) t -> p c t", p=P),
            )
            nc.gpsimd.dma_start(
                xb[:],
                xTq[:, (qp + 2) * NT : (qp + 3) * NT].rearrange(
                    "(c p) t -> p c t", p=P
                ),
            )
            for dst, lo, hi in ((qq1, xa, xb), (qq2, xb, xa)):
                pq = pps.tile([P, 2 * NT], f32, tag="sc", name=f"pq_{qp}_{dst is qq2}")
                for e in range(NE):
                    nc.tensor.matmul(
                        pq[0:D, 0:NT], wq_sb[:, e, :], lo[:, e, :],
                        start=(e == 0), stop=False,
                    )
                    nc.tensor.matmul(
                        pq[D:P, NT : 2 * NT], wq_sb[:, e, :], hi[:, e, :],
                        start=(e == 0), stop=False,
                    )
                nc.tensor.matmul(
                    pq[0:D, 0:NT], bq_sb[:], ones_sb[:], start=False, stop=True
                )
                nc.tensor.matmul(
                    pq[D:P, NT : 2 * NT], bq_sb[:], ones_sb[:], start=False, stop=True
                )
                nc.vector.tensor_copy(
                    out=dst[0:D, qp * NT : (qp + 1) * NT], in_=pq[0:D, 0:NT]
                )
                nc.vector.tensor_copy(
                    out=dst[D:P, qp * NT : (qp + 1) * NT], in_=pq[D:P, NT : 2 * NT]
                )

        # ---- per token-pair: K/V projection, V transposes, attention ----
        pav = [None] * NQC

        for p in range(4):  # token pair (512p, 512p+2048)
            xa = xgp.tile([P, NE, NT], f16, tag="xg")
            xb = xgp.tile([P, NE, NT], f16, tag="xg")
            nc.gpsimd.dma_start(
                xa[:],
                xT[:, p * NT : (p + 1) * NT].rearrange("(c p) t -> p c t", p=P),
            )
            nc.gpsimd.dma_start(
                xb[:],
                xT[:, (p + 4) * NT : (p + 5) * NT].rearrange(
                    "(c p) t -> p c t", p=P
                ),
            )
            pk = pps.tile([P, 2 * NT], f32, tag="sc")
            pv = pps.tile([P, 2 * NT], f32, tag="sc")
            for e in range(NE):
                nc.tensor.matmul(
                    pk[0:D, 0:NT], wk_sb[:, e, :], xa[:, e, :],
                    start=(e == 0), stop=False,
                )
                nc.tensor.matmul(
                    pk[D:P, NT : 2 * NT], wk_sb[:, e, :], xb[:, e, :],
                    start=(e == 0), stop=False,
                )
            nc.tensor.matmul(
                pk[0:D, 0:NT], bk_sb[:], ones_sb[:], start=False, stop=True
            )
            nc.tensor.matmul(
                pk[D:P, NT : 2 * NT], bk_sb[:], ones_sb[:], start=False, stop=True
            )
            for e in range(NE):
                nc.tensor.matmul(
                    pv[0:D, 0:NT], wv_sb[:, e, :], xa[:, e, :],
                    start=(e == 0), stop=False,
                )
                nc.tensor.matmul(
                    pv[D:P, NT : 2 * NT], wv_sb[:, e, :], xb[:, e, :],
                    start=(e == 0), stop=False,
                )
            nc.tensor.matmul(
                pv[0:D, 0:NT], bv_sb[:], ones_sb[:], start=False, stop=True
            )
            nc.tensor.matmul(
                pv[D:P, NT : 2 * NT], bv_sb[:], ones_sb[:], start=False, stop=True
            )
            nc.scalar.copy(out=kk[0:D, p * NT : (p + 1) * NT], in_=pk[0:D, 0:NT])
            nc.scalar.copy(
                out=kk[D:P, p * NT : (p + 1) * NT], in_=pk[D:P, NT : 2 * NT]
            )
            nc.vector.tensor_copy(
                out=vv[0:D, p * NT : (p + 1) * NT], in_=pv[0:D, 0:NT]
            )
            nc.vector.tensor_copy(
                out=vv[D:P, p * NT : (p + 1) * NT], in_=pv[D:P, NT : 2 * NT]
            )

            # V transposes for this pair: chunks 4p..4p+3 (lo), 16+4p.. (hi)
            for cc in range(4 * p, 4 * p + 4):
                pvt = pps.tile([P, P], f16, tag="sc")
                nc.tensor.transpose(
                    out=pvt[:, :],
                    in_=vv[:, cc * P : (cc + 1) * P],
                    identity=id_sb[:, :],
                )
                w0 = cc * (D + 1)
                w1 = (16 + cc) * (D + 1)
                nc.vector.tensor_copy(out=vaug[:, w0 : w0 + D], in_=pvt[:, 0:D])
                nc.vector.tensor_copy(
                    out=vaug[:, w1 : w1 + D], in_=pvt[:, D : 2 * D]
                )

            # ---- attention g-blocks for this pair ----
            for g in range(4 * p, 4 * p + 4):
                for qc in range(NQC):
                    if pav[qc] is None:
                        avtile = pps.tile([D + 1, NT], f32, tag="av", bufs=4)
                        pav[qc] = avtile
                    # rhs halves for Tq chunk qc
                    if qc < 2:
                        rhs_a = qq1[0:D, qc * NT : (qc + 1) * NT]
                        rhs_b = qq2[D:P, qc * NT : (qc + 1) * NT]
                    else:
                        rhs_a = qq2[0:D, (qc - 2) * NT : (qc - 1) * NT]
                        rhs_b = qq1[D:P, (qc - 2) * NT : (qc - 1) * NT]
                    psc = pps.tile([P, 2 * NT], f32, tag="sc")
                    nc.tensor.matmul(
                        psc[:, 0:NT],
                        kk[0:D, g * P : (g + 1) * P],
                        rhs_a,
                        start=True, stop=True,
                    )
                    nc.tensor.matmul(
                        psc[:, NT : 2 * NT],
                        kk[D:P, g * P : (g + 1) * P],
                        rhs_b,
                        start=True, stop=True,
                    )
                    ex = expp.tile([P, 2 * NT], f16, tag="ex")
                    nc.scalar.activation(ex[:], psc[:], EXP, scale=0.125)
                    w0 = g * (D + 1)
                    w1 = (16 + g) * (D + 1)
                    nc.tensor.matmul(
                        pav[qc][:, :],
                        vaug[:, w0 : w0 + D + 1],
                        ex[:, 0:NT],
                        start=(g == 0), stop=False,
                        skip_group_check=True,
                    )
                    nc.tensor.matmul(
                        pav[qc][:, :],
                        vaug[:, w1 : w1 + D + 1],
                        ex[:, NT : 2 * NT],
                        start=False, stop=(g == 15),
                        skip_group_check=True,
                    )

        for qc in range(NQC):
            osb = outp.tile([D + 1, NT], f32, tag="osb")
            nc.vector.tensor_copy(out=osb[:], in_=pav[qc][:])
            nc.sync.dma_start(out[:, qc * NT : (qc + 1) * NT], osb[:])


def kernel(hidden_state, q_w, q_b, k_w, k_b, v_w, v_b):
    global LAST_RESULTS
    from concourse.bass_utils import run_bass_kernel_spmd

    hidden_state = np.asarray(hidden_state, dtype=np.float32)
    q_w = np.asarray(q_w, dtype=np.float32)
    q_b = np.asarray(q_b, dtype=np.float32)
    k_w = np.asarray(k_w, dtype=np.float32)
    k_b = np.asarray(k_b, dtype=np.float32)
    v_w = np.asarray(v_w, dtype=np.float32)
    v_b = np.asarray(v_b, dtype=np.float32)

    B, S, E = hidden_state.shape
    assert (B, S, E) == (4, SEQ, EMBED)

    if "nc" not in _CACHE:
        _CACHE["nc"] = _build_bass()
    nc = _CACHE["nc"]

    ident = np.eye(P, dtype=np.float32)
    shared = {
        "wq": q_w,
        "wk": k_w,
        "wv": v_w,
        "bq": q_b.reshape(1, D).astype(np.float32),
        "bk": k_b.reshape(1, D).astype(np.float32),
        "bv": v_b.reshape(1, D).astype(np.float32),
        "ident": ident,
    }
    xTs = [np.ascontiguousarray(hidden_state[b].T) for b in range(B)]
    in_maps = []
    for c in range(N_CORES):
        b, h = c // 2, c % 2
        m = dict(shared)
        m["xT"] = xTs[b]
        m["xTq"] = np.ascontiguousarray(xTs[b][:, h * TQ : (h + 1) * TQ])
        in_maps.append(m)

    trace = bool(int(os.environ.get("KERNEL_TRACE", "0")))
    res = run_bass_kernel_spmd(nc, in_maps, list(range(N_CORES)), trace=trace)
    LAST_RESULTS = res

    outp = np.empty((B, S, D), dtype=np.float32)
    for c in range(N_CORES):
        b, h = c // 2, c % 2
        r = res.results[c]["out"]  # [65, 2048]
        outp[b, h * TQ : (h + 1) * TQ, :] = (r[:D] / r[D : D + 1]).T
    return outp


# revision 14
# speedup vs baseline: 160.8455x; 1.4303x over previous
"""AttentionHead kernel for Trainium2, 8 NeuronCores.

Sharding: core c -> (batch b = c//2, query-half h = c%2).
Each core computes K/V projections over the full 4096-token sequence of its
batch and Q projections + attention for its 2048-token query half.

Host-side prep: hidden_state[b] is transposed to xT = [EMBED, SEQ] so the
contraction dim (EMBED) lands on SBUF partitions with no on-chip transposes.

v2: PE array packing + proj/attention interleaving.
 - Projections are col-paired (tile_position (0,0)/(0,64)): one matmul pair
   computes two 64-wide output blocks concurrently on the two column halves
   of the PE array.  K^T/V^T land as [128, 2048] tiles whose rows 0:64 hold
   token chunks 0:2048 and rows 64:128 hold chunks 2048:4096.
 - Scores are row-paired (tile_position (0,0)/(64,0)): contraction is D=64,
   so two independent S^T chunk matmuls share the PE array's row halves.
 - Emission interleaves: token-pair p's K/V projection + V transposes are
   followed immediately by attention g-blocks 4p..4p+3, so attention starts
   as soon as the first 4MB of activations has landed; DMA and projections
   hide behind the ACT-bound exp stream.

scores: S^T[tk, tq] = (K^T chunk).T @ Q^T   (f32 psum, pairs -> [128, 1024])
exp on ScalarE, fused 1/sqrt(64) scale (no max subtraction: |scores/8| < ~3)
AV: out^T_aug[65, tq] += vaug[chunk].T @ expS^T chunk  (vaug has ones column,
row 64 accumulates the softmax denominator).
Output per core: [65, 2048] f32; host divides rows 0:64 by row 64, transposes.

All matmul operands fp16 (SWDGE casts f32->f16 in flight); psum f32.
"""

import os
import numpy as np

EMBED = 1024
SEQ = 4096
TQ = 2048  # query tokens per core
D = 64
NT = 512  # token chunk (free dim) for projections
P = 128
N_CORES = 8

_CACHE = {}
LAST_RESULTS = None


def _build_bass(repeats=1):
    import concourse.bass as bass
    import concourse.mybir as mybir
    import concourse.tile as tile
    from concourse import bacc

    f32 = mybir.dt.float32
    f16 = mybir.dt.float16

    nc = bacc.Bacc("TRN2", target_bir_lowering=False, debug=False)

    xT = nc.dram_tensor("xT", [EMBED, SEQ], f16, kind="ExternalInput").ap()
    xTq = nc.dram_tensor("xTq", [EMBED, TQ], f16, kind="ExternalInput").ap()
    wq = nc.dram_tensor("wq", [EMBED, D], f16, kind="ExternalInput").ap()
    wk = nc.dram_tensor("wk", [EMBED, D], f16, kind="ExternalInput").ap()
    wv = nc.dram_tensor("wv", [EMBED, D], f16, kind="ExternalInput").ap()
    bq = nc.dram_tensor("bq", [1, D], f16, kind="ExternalInput").ap()
    bk = nc.dram_tensor("bk", [1, D], f16, kind="ExternalInput").ap()
    bv = nc.dram_tensor("bv", [1, D], f16, kind="ExternalInput").ap()
    ident = nc.dram_tensor("ident", [P, P], f16, kind="ExternalInput").ap()
    out = nc.dram_tensor("out", [D + 1, TQ], f32, kind="ExternalOutput").ap()

    NE = EMBED // P  # 8 embed chunks

    with tile.TileContext(nc) as tc:
        with tc.tile_pool(name="const", bufs=1) as const:
            wq_sb = const.tile([P, NE, D], f16, tag="wq")
            wk_sb = const.tile([P, NE, D], f16, tag="wk")
            wv_sb = const.tile([P, NE, D], f16, tag="wv")
            nc.sync.dma_start(wq_sb[:], wq.rearrange("(c p) d -> p c d", p=P))
            nc.sync.dma_start(wk_sb[:], wk.rearrange("(c p) d -> p c d", p=P))
            nc.sync.dma_start(wv_sb[:], wv.rearrange("(c p) d -> p c d", p=P))
            bq_sb = const.tile([1, D], f16, tag="bq")
            bk_sb = const.tile([1, D], f16, tag="bk")
            bv_sb = const.tile([1, D], f16, tag="bv")
            nc.sync.dma_start(bq_sb[:], bq[:])
            nc.sync.dma_start(bk_sb[:], bk[:])
            nc.sync.dma_start(bv_sb[:], bv[:])
            id_sb = const.tile([P, P], f16, tag="ident")
            nc.sync.dma_start(id_sb[:], ident[:])
            ones_sb = const.tile([1, NT], f16, tag="ones")
            nc.gpsimd.memset(ones_sb[:], 1.0)

            # rows 0:64 = token chunks [0, 2048); rows 64:128 = [2048, 4096)
            kk = const.tile([P, SEQ // 2], f16, tag="kk")
            vv = const.tile([P, SEQ // 2], f16, tag="vv")
            # qq1 rows 0:64 = q chunks [0,1024); rows 64:128 = [1024, 2048)
            # qq2 = qq1 with the halves swapped (partition-shift DMA)
            qq1 = const.tile([P, TQ // 2], f16, tag="qq1")
            qq2 = const.tile([P, TQ // 2], f16, tag="qq2")
            NKC = SEQ // P  # 32 key chunks
            vaug = const.tile([P, NKC * (D + 1)], f16, tag="vaug")
            nc.gpsimd.memset(
                vaug[:].rearrange("p (c w) -> p c w", w=D + 1)[:, :, D : D + 1], 1.0
            )

            for _rep in range(repeats):
                _kernel_body(
                    nc, tc, mybir,
                    xT, xTq, out,
                    wq_sb, wk_sb, wv_sb, bq_sb, bk_sb, bv_sb, id_sb, ones_sb,
                    kk, vv, qq1, qq2, vaug,
                )

    nc.compile()
    return nc


def _kernel_body(
    nc, tc, mybir,
    xT, xTq, out,
    wq_sb, wk_sb, wv_sb, bq_sb, bk_sb, bv_sb, id_sb, ones_sb,
    kk, vv, qq1, qq2, vaug,
):
    f32 = mybir.dt.float32
    f16 = mybir.dt.float16
    EXP = mybir.ActivationFunctionType.Exp
    NE = EMBED // P
    NKC = SEQ // P
    NQC = TQ // NT  # 4 query chunks of 512

    with (
        tc.tile_pool(name="xg", bufs=4) as xgp,
        tc.tile_pool(name="psum", bufs=2, space="PSUM") as pps,
        tc.tile_pool(name="expp", bufs=3) as expp,
        tc.tile_pool(name="outp", bufs=2) as outp,
    ):
        # ---- Q projections first (gate all attention) ----
        # qq1 cols [512qp:+512]: rows 0:64 = Q chunk 2qp, rows 64:128 = 2qp+1
        # qq2: same cols, chunks swapped
        for qp in range(2):  # pair q-group 2qp with 2qp+1
            xa = xgp.tile([P, NE, NT], f16, tag="xg")
            xb = xgp.tile([P, NE, NT], f16, tag="xg")
            nc.sync.dma_start(
                xa[:],
                xTq[:, 2 * qp * NT : (2 * qp + 1) * NT].rearrange(
                    "(c p) t -> p c t", p=P
                ),
            )
            nc.sync.dma_start(
                xb[:],
                xTq[:, (2 * qp + 1) * NT : (2 * qp + 2) * NT].rearrange(
                    "(c p) t -> p c t", p=P
                ),
            )
            for dst, lo, hi in ((qq1, xa, xb), (qq2, xb, xa)):
                pq = pps.tile([P, 2 * NT], f32, tag="sc", name=f"pq_{qp}_{dst is qq2}")
                for e in range(NE):
                    nc.tensor.matmul(
                        pq[0:D, 0:NT], wq_sb[:, e, :], lo[:, e, :],
                        start=(e == 0), stop=False,
                    )
                    nc.tensor.matmul(
                        pq[D:P, NT : 2 * NT], wq_sb[:, e, :], hi[:, e, :],
                        start=(e == 0), stop=False,
                    )
                nc.tensor.matmul(
                    pq[0:D, 0:NT], bq_sb[:], ones_sb[:], start=False, stop=True
                )
                nc.tensor.matmul(
                    pq[D:P, NT : 2 * NT], bq_sb[:], ones_sb[:], start=False, stop=True
                )
                nc.vector.tensor_copy(
                    out=dst[0:D, qp * NT : (qp + 1) * NT], in_=pq[0:D, 0:NT]
                )
                nc.vector.tensor_copy(
                    out=dst[D:P, qp * NT : (qp + 1) * NT], in_=pq[D:P, NT : 2 * NT]
                )

        # ---- per token-pair: K/V projection, V transposes, attention ----
        pav = [None] * NQC

        for p in range(4):  # token pair (512p, 512p+2048)
            xa = xgp.tile([P, NE, NT], f16, tag="xg")
            xb = xgp.tile([P, NE, NT], f16, tag="xg")
            nc.sync.dma_start(
                xa[:],
                xT[:, p * NT : (p + 1) * NT].rearrange("(c p) t -> p c t", p=P),
            )
            nc.sync.dma_start(
                xb[:],
                xT[:, (p + 4) * NT : (p + 5) * NT].rearrange(
                    "(c p) t -> p c t", p=P
                ),
            )
            pk = pps.tile([P, 2 * NT], f32, tag="sc")
            pv = pps.tile([P, 2 * NT], f32, tag="sc")
            for e in range(NE):
                nc.tensor.matmul(
                    pk[0:D, 0:NT], wk_sb[:, e, :], xa[:, e, :],
                    start=(e == 0), stop=False,
                )
                nc.tensor.matmul(
                    pk[D:P, NT : 2 * NT], wk_sb[:, e, :], xb[:, e, :],
                    start=(e == 0), stop=False,
                )
            nc.tensor.matmul(
                pk[0:D, 0:NT], bk_sb[:], ones_sb[:], start=False, stop=True
            )
            nc.tensor.matmul(
                pk[D:P, NT : 2 * NT], bk_sb[:], ones_sb[:], start=False, stop=True
            )
            for e in range(NE):
                nc.tensor.matmul(
                    pv[0:D, 0:NT], wv_sb[:, e, :], xa[:, e, :],
                    start=(e == 0), stop=False,
                )
                nc.tensor.matmul(
                    pv[D:P, NT : 2 * NT], wv_sb[:, e, :], xb[:, e, :],
                    start=(e == 0), stop=False,
                )
            nc.tensor.matmul(
                pv[0:D, 0:NT], bv_sb[:], ones_sb[:], start=False, stop=True
            )
            nc.tensor.matmul(
                pv[D:P, NT : 2 * NT], bv_sb[:], ones_sb[:], start=False, stop=True
            )
            nc.scalar.copy(out=kk[0:D, p * NT : (p + 1) * NT], in_=pk[0:D, 0:NT])
            nc.scalar.copy(
                out=kk[D:P, p * NT : (p + 1) * NT], in_=pk[D:P, NT : 2 * NT]
            )
            nc.vector.tensor_copy(
                out=vv[0:D, p * NT : (p + 1) * NT], in_=pv[0:D, 0:NT]
            )
            nc.vector.tensor_copy(
                out=vv[D:P, p * NT : (p + 1) * NT], in_=pv[D:P, NT : 2 * NT]
            )

            # V transposes for this pair: chunks 4p..4p+3 (lo), 16+4p.. (hi)
            for cc in range(4 * p, 4 * p + 4):
                pvt = pps.tile([P, P], f16, tag="sc")
                nc.tensor.transpose(
                    out=pvt[:, :],
                    in_=vv[:, cc * P : (cc + 1) * P],
                    identity=id_sb[:, :],
                )
                w0 = cc * (D + 1)
                w1 = (16 + cc) * (D + 1)
                nc.vector.tensor_copy(out=vaug[:, w0 : w0 + D], in_=pvt[:, 0:D])
                nc.vector.tensor_copy(
                    out=vaug[:, w1 : w1 + D], in_=pvt[:, D : 2 * D]
                )

            # ---- attention g-blocks for this pair ----
            for g in range(4 * p, 4 * p + 4):
                for qc in range(NQC):
                    if pav[qc] is None:
                        avtile = pps.tile([D + 1, NT], f32, tag="av", bufs=4)
                        pav[qc] = avtile
                    # rhs halves for Tq chunk qc (col group qc//2)
                    c0 = (qc // 2) * NT
                    if qc % 2 == 0:
                        rhs_a = qq1[0:D, c0 : c0 + NT]
                        rhs_b = qq2[D:P, c0 : c0 + NT]
                    else:
                        rhs_a = qq2[0:D, c0 : c0 + NT]
                        rhs_b = qq1[D:P, c0 : c0 + NT]
                    psc = pps.tile([P, 2 * NT], f32, tag="sc")
                    nc.tensor.matmul(
                        psc[:, 0:NT],
                        kk[0:D, g * P : (g + 1) * P],
                        rhs_a,
                        start=True, stop=True,
                    )
                    nc.tensor.matmul(
                        psc[:, NT : 2 * NT],
                        kk[D:P, g * P : (g + 1) * P],
                        rhs_b,
                        start=True, stop=True,
                    )
                    ex = expp.tile([P, 2 * NT], f16, tag="ex")
                    nc.scalar.activation(ex[:], psc[:], EXP, scale=0.125)
                    w0 = g * (D + 1)
                    w1 = (16 + g) * (D + 1)
                    nc.tensor.matmul(
                        pav[qc][:, :],
                        vaug[:, w0 : w0 + D + 1],
                        ex[:, 0:NT],
                        start=(g == 0), stop=False,
                        skip_group_check=True,
                    )
                    nc.tensor.matmul(
                        pav[qc][:, :],
                        vaug[:, w1 : w1 + D + 1],
                        ex[:, NT : 2 * NT],
                        start=False, stop=(g == 15),
                        skip_group_check=True,
                    )

        for qc in range(NQC):
            osb = outp.tile([D + 1, NT], f32, tag="osb")
            nc.vector.tensor_copy(out=osb[:], in_=pav[qc][:])
            nc.sync.dma_start(out[:, qc * NT : (qc + 1) * NT], osb[:])


def kernel(hidden_state, q_w, q_b, k_w, k_b, v_w, v_b):
    global LAST_RESULTS
    from concourse.bass_utils import run_bass_kernel_spmd

    hidden_state = np.asarray(hidden_state, dtype=np.float32)
    q_w = np.asarray(q_w, dtype=np.float32)
    q_b = np.asarray(q_b, dtype=np.float32)
    k_w = np.asarray(k_w, dtype=np.float32)
    k_b = np.asarray(k_b, dtype=np.float32)
    v_w = np.asarray(v_w, dtype=np.float32)
    v_b = np.asarray(v_b, dtype=np.float32)

    B, S, E = hidden_state.shape
    assert (B, S, E) == (4, SEQ, EMBED)

    if "nc" not in _CACHE:
        _CACHE["nc"] = _build_bass()
    nc = _CACHE["nc"]

    ident = np.eye(P, dtype=np.float16)
    shared = {
        "wq": q_w.astype(np.float16),
        "wk": k_w.astype(np.float16),
        "wv": v_w.astype(np.float16),
        "bq": q_b.reshape(1, D).astype(np.float16),
        "bk": k_b.reshape(1, D).astype(np.float16),
        "bv": v_b.reshape(1, D).astype(np.float16),
        "ident": ident,
    }
    xTs = [np.ascontiguousarray(hidden_state[b].T.astype(np.float16)) for b in range(B)]
    in_maps = []
    for c in range(N_CORES):
        b, h = c // 2, c % 2
        m = dict(shared)
        m["xT"] = xTs[b]
        m["xTq"] = np.ascontiguousarray(xTs[b][:, h * TQ : (h + 1) * TQ])
        in_maps.append(m)

    trace = bool(int(os.environ.get("KERNEL_TRACE", "0")))
    res = run_bass_kernel_spmd(nc, in_maps, list(range(N_CORES)), trace=trace)
    LAST_RESULTS = res

    outp = np.empty((B, S, D), dtype=np.float32)
    for c in range(N_CORES):
        b, h = c // 2, c % 2
        r = res.results[c]["out"]  # [65, 2048]
        outp[b, h * TQ : (h + 1) * TQ, :] = (r[:D] / r[D : D + 1]).T
    return outp
